# revision 6
# baseline (speedup 1.0000x reference)
"""Trainium2 Bass kernel for nn_Decoder_Model (dense transformer decoder layer).

Sharding: data-parallel over batch (8 batches -> 8 cores). The three global
layernorms (normalized over ALL elements of the [B,S,D] tensor) need cross-core
scalar stats: each core computes local sum/sumsq, an 8-float AllReduce merges
them. AllReduce latency is hidden behind the next phase's matmuls using the
affine trick: norm(x)@W.T = (x@W.T)*rstd + per-channel-fix, so the big matmuls
run on raw x while stats are in flight and only a cheap fixup pass waits.

Layout: activations feature-major [feature, token] in SBUF so every matmul
contraction has its axis on partitions. V is produced token-major with an
appended ones column per head, so probs@V also yields the softmax denominator
for free. Matmuls use float32r (full-rate fp32, ~1e-4 rel err). Weights are
transposed on demand through small rotating block buffers to fit SBUF.
"""
import sys

import numpy as np

sys.path.insert(0, "/opt/trn_rl_repo")

import concourse.bass as bass  # noqa: E402,F401
import concourse.mybir as mybir  # noqa: E402
import concourse.tile as tile  # noqa: E402
from concourse import bacc  # noqa: E402
from concourse import bass_utils  # noqa: E402
from concourse.masks import make_identity  # noqa: E402

F32 = mybir.dt.float32
F32R = mybir.dt.float32r
AF = mybir.ActivationFunctionType
OP = mybir.AluOpType

B, S, D, H, DK, FF = 8, 1024, 512, 8, 64, 2048
TT = S // 128   # 8 token tiles
DT = D // 128   # 4 feature tiles
FT = FF // 128  # 16 ffn tiles
TH = S // 512   # 2 matmul free-dim halves
N_CORES = 8
NTOT = float(B * S * D)
EPS = 1e-5

WNAMES = ["wq_m", "wk_m", "wv_m", "wo_m", "wq_c", "wk_c", "wv_c", "wo_c"]
BNAMES = ["bq_m", "bk_m", "bv_m", "bo_m", "bq_c", "bk_c", "bv_c", "bo_c"]

# self-attn causal chunking per k-tile kt over the q axis:
# (masked_chunk_start, masked_chunk_width, [(clean_start, clean_width), ...])
CAUSAL_CHUNKS = {
    0: (0, 256, [(256, 512), (768, 256)]),
    1: (128, 256, [(384, 384), (768, 256)]),
    2: (256, 256, [(512, 512)]),
    3: (384, 256, [(640, 384)]),
    4: (512, 256, [(768, 256)]),
    5: (640, 384, []),
    6: (768, 256, []),
    7: (896, 128, []),
}


def build_nc():
    nc = bacc.Bacc("TRN2", target_bir_lowering=False, debug=False,
                   enable_asserts=False, num_devices=N_CORES)
    inp = {}
    inp["data_dec"] = nc.dram_tensor("data_dec", [S, D], F32,
                                     kind="ExternalInput").ap()
    inp["encoder_out"] = nc.dram_tensor("encoder_out", [S, D], F32,
                                        kind="ExternalInput").ap()
    for w in WNAMES:
        inp[w] = nc.dram_tensor(w, [D, D], F32, kind="ExternalInput").ap()
    for b in BNAMES:
        inp[b] = nc.dram_tensor(b, [D], F32, kind="ExternalInput").ap()
    inp["wf1"] = nc.dram_tensor("wf1", [FF, D], F32, kind="ExternalInput").ap()
    inp["bf1"] = nc.dram_tensor("bf1", [FF], F32, kind="ExternalInput").ap()
    inp["wf2"] = nc.dram_tensor("wf2", [D, FF], F32, kind="ExternalInput").ap()
    inp["bf2"] = nc.dram_tensor("bf2", [D], F32, kind="ExternalInput").ap()
    out_d = nc.dram_tensor("out", [S, D], F32, kind="ExternalOutput").ap()

    with tile.TileContext(nc) as tc:
        build_body(nc, tc, inp, out_d)
    nc.finalize()
    return nc


def build_body(nc, tc, inp, out_d):
    import contextlib
    ctx = contextlib.ExitStack()
    with ctx:
        sb = ctx.enter_context(tc.tile_pool(name="sb", bufs=1))
        stg = ctx.enter_context(tc.tile_pool(name="stg", bufs=2))
        wp = ctx.enter_context(tc.tile_pool(name="wp", bufs=1))
        cp = ctx.enter_context(tc.tile_pool(name="cp", bufs=2))
        dram = ctx.enter_context(tc.tile_pool(name="dram", bufs=1, space="DRAM"))
        ps_a = ctx.enter_context(tc.tile_pool(name="ps_a", bufs=3, space="PSUM"))
        ps_b = ctx.enter_context(tc.tile_pool(name="ps_b", bufs=2, space="PSUM"))
        ps_pv = ctx.enter_context(tc.tile_pool(name="ps_pv", bufs=3, space="PSUM"))

        def psA():
            return ps_a.tile([128, 512], F32, tag="A", name="pA")

        def psB():
            return ps_b.tile([128, 512], F32, tag="B", name="pB")

        def wstage():
            return stg.tile([128, 512], F32, tag="wstage", name="wstg")

        ident = sb.tile([128, 128], F32, tag="ident")
        make_identity(nc, ident[:])
        onesf = sb.tile([128, 1], F32, tag="onesf")
        nc.vector.memset(onesf[:], 1.0)

        # binary causal mask (keep q >= k within the diagonal chunk)
        mask_f = wstage()
        nc.vector.memset(mask_f[:], 1.0)
        nc.gpsimd.affine_select(out=mask_f[:], in_=mask_f[:],
                                compare_op=OP.is_ge, fill=0.0, base=0,
                                channel_multiplier=-1, pattern=[[1, 512]])
        mask_r = sb.tile([128, 512], F32R, tag="mask_r")
        nc.vector.tensor_copy(mask_r[:], mask_f[:])

        # ---- transpose helper: [128,128] f32 sbuf block -> f32r dest ----
        def transpose_block(dst_ap, src_ap):
            pt = psA()
            nc.tensor.transpose(pt[:, :128], src_ap, ident[:])
            nc.scalar.copy(dst_ap, pt[:, :128])

        # ---- load x / enc, transpose to feature-major ----
        x_T = sb.tile([128, DT, S], F32R, tag="g_x")       # shared tag: later nmm
        enc_T = sb.tile([128, DT, S], F32R, tag="g_enc")   # shared tag: later r2
        for src_d, trn in ((inp["data_dec"], x_T), (inp["encoder_out"], enc_T)):
            for tt in range(TT):
                stage = wstage()
                nc.sync.dma_start(
                    stage[:], src_d.rearrange("(tt p) d -> p tt d", p=128)[:, tt])
                for dd in range(DT):
                    transpose_block(trn[:, dd, tt * 128:(tt + 1) * 128],
                                    stage[:, dd * 128:(dd + 1) * 128])

        # ---- biases ----
        bias = {}
        for b in BNAMES + ["bf2"]:
            t = sb.tile([128, DT], F32, tag=f"{b}_sb")
            nc.sync.dma_start(t[:], inp[b].rearrange("(t p) -> p t", p=128))
            bias[b] = t
        bf1_sb = sb.tile([128, FT], F32, tag="bf1_sb")
        nc.sync.dma_start(bf1_sb[:], inp["bf1"].rearrange("(t p) -> p t", p=128))
        bv_full = {}
        for b in ("bv_m", "bv_c"):
            row = wstage()
            nc.sync.dma_start(row[0:1, :], inp[b][None, :])
            full = sb.tile([128, D], F32, tag=f"{b}_full")
            nc.gpsimd.partition_broadcast(full[:], row[0:1, :])
            bv_full[b] = full

        # column sums of wq_c (norm1 affine fix) and wf1 (norm2 fix)
        wsum_qc = sb.tile([128, DT], F32, tag="wsum_qc")
        for ot in range(DT):
            stage = wstage()
            nc.sync.dma_start(
                stage[:], inp["wq_c"].rearrange("(t p) i -> p t i", p=128)[:, ot])
            nc.vector.reduce_sum(wsum_qc[:, ot:ot + 1], stage[:],
                                 axis=mybir.AxisListType.X)
        wsum_f1 = sb.tile([128, FT], F32, tag="wsum_f1")
        for ot in range(FT):
            stage = wstage()
            nc.sync.dma_start(
                stage[:], inp["wf1"].rearrange("(t p) i -> p t i", p=128)[:, ot])
            nc.vector.reduce_sum(wsum_f1[:, ot:ot + 1], stage[:],
                                 axis=mybir.AxisListType.X)

        # ---- weight block loaders (on-demand transpose) ----
        def load_wblk(wname, ot, n_in_tiles=DT):
            """Natural row-tile ot of W -> transposed blocks [128, n_in, 128]."""
            stage = wstage()
            nc.sync.dma_start(
                stage[:], inp[wname].rearrange("(t p) i -> p t i", p=128)[:, ot])
            blk = wp.tile([128, DT, 128], F32R, tag="wblk", name="wblk", bufs=2)
            for ki in range(n_in_tiles):
                transpose_block(blk[:, ki], stage[:, ki * 128:(ki + 1) * 128])
            return blk

        def load_wblk_ff2(dd):
            """wf2 natural row-tile dd ([128, 2048]) -> [128, FT, 128] blocks."""
            blk = wp.tile([128, FT, 128], F32R, tag="wblk2", name="wblk2")
            for piece in range(4):
                stage = wstage()
                nc.sync.dma_start(
                    stage[:],
                    inp["wf2"].rearrange("(t p) i -> p t i", p=128)
                    [:, dd, piece * 512:(piece + 1) * 512])
                for j in range(4):
                    transpose_block(blk[:, piece * 4 + j],
                                    stage[:, j * 128:(j + 1) * 128])
            return blk

        def load_wvT(wname):
            """Full W^T [128, DT(ki), 512(dout)] for the v projection rhs."""
            wvT = wp.tile([128, DT, 512], F32R, tag="wvT", name="wvT")
            for ot in range(DT):
                stage = wstage()
                nc.sync.dma_start(
                    stage[:], inp[wname].rearrange("(t p) i -> p t i", p=128)[:, ot])
                for ki in range(DT):
                    transpose_block(wvT[:, ki, ot * 128:(ot + 1) * 128],
                                    stage[:, ki * 128:(ki + 1) * 128])
            return wvT

        # ---- projection helpers ----
        def project_fm(wname, src_T, out_tile, bias_tile=None,
                       fix_tile=None, scale_ap=None):
            """Feature-major projection: out[:, dd, :] = act(W^T @ src)."""
            for dd in range(DT):
                blk = load_wblk(wname, dd)
                for th in range(TH):
                    pt = psB()
                    for ki in range(DT):
                        nc.tensor.matmul(pt[:], blk[:, ki],
                                         src_T[:, ki, th * 512:(th + 1) * 512],
                                         start=(ki == 0), stop=(ki == DT - 1))
                    dst = out_tile[:, dd, th * 512:(th + 1) * 512]
                    if fix_tile is not None:
                        nc.scalar.activation(dst, pt[:], AF.Identity,
                                             bias=fix_tile[:, dd:dd + 1],
                                             scale=scale_ap[:])
                    else:
                        nc.scalar.activation(dst, pt[:], AF.Identity,
                                             bias=bias_tile[:, dd:dd + 1])

        def project_v(wname, bname, src_T, v_tok):
            """Token-major v with per-head ones column: v_tok [128, TT, H*65]."""
            wvT = load_wvT(wname)
            ones_view = v_tok[:, :, :].rearrange(
                "p t (h c) -> p t h c", c=65)[:, :, :, 64]
            nc.vector.tensor_copy(
                ones_view, onesf[:, 0:1, None].to_broadcast([128, TT, H]))
            for tt in range(TT):
                pt = psB()
                for ki in range(DT):
                    nc.tensor.matmul(pt[:], src_T[:, ki, tt * 128:(tt + 1) * 128],
                                     wvT[:, ki],
                                     start=(ki == 0), stop=(ki == DT - 1))
                dstv = v_tok[:, tt].rearrange("p (h c) -> p h c", c=65)[:, :, 0:64]
                nc.vector.tensor_tensor(
                    dstv, pt[:].rearrange("p (h c) -> p h c", c=64),
                    bv_full[bname][:].rearrange("p (h c) -> p h c", c=64),
                    OP.add)

        def attention(q_T, k_T, v_tok, attn_T, causal):
            for h in range(H):
                dt_, base = h // 2, (h % 2) * 64
                q_h = q_T[base:base + 64, dt_]
                k_h = k_T[base:base + 64, dt_]
                pv = {qh: ps_pv.tile([128, 512], F32, tag="PV", name="pPV")
                      for qh in range(TH)}
                for kt in range(TT):
                    pr = cp.tile([128, S], F32R, tag="probs", name="probs")
                    if causal:
                        m0, mw, clean = CAUSAL_CHUNKS[kt]
                        chunks = [(m0, mw, True)] + [(c0, cw, False)
                                                     for (c0, cw) in clean]
                    else:
                        chunks = [(0, 512, False), (512, 512, False)]
                    for (c0, cw, masked) in chunks:
                        st = psA()
                        nc.tensor.matmul(st[:, :cw],
                                         k_h[:, kt * 128:(kt + 1) * 128],
                                         q_h[:, c0:c0 + cw],
                                         start=True, stop=True)
                        nc.scalar.activation(pr[:, c0:c0 + cw], st[:, :cw],
                                             AF.Exp, scale=1.0 / 32.0)
                        if masked:
                            nc.vector.tensor_tensor(pr[:, c0:c0 + cw],
                                                    pr[:, c0:c0 + cw],
                                                    mask_r[:, 0:cw], OP.mult)
                    # PV contributions of this kt
                    v_h = v_tok[:, kt, h * 65:(h + 1) * 65]
                    for qh in range(TH):
                        if causal and qh == 0 and kt > 3:
                            continue
                        if causal:
                            off = max(0, (kt - qh * 4) * 128)
                            last = (kt == 3) if qh == 0 else (kt == 7)
                        else:
                            off, last = 0, (kt == 7)
                        nc.tensor.matmul(
                            pv[qh][:65, off:512], v_h,
                            pr[:, qh * 512 + off:(qh + 1) * 512],
                            start=(kt == 0), stop=last)
                # normalize by the rowsum living in row 64 of pv
                for qh in range(TH):
                    rec = cp.tile([1, 512], F32, tag="rsrec", name="rec", bufs=1)
                    nc.vector.reciprocal(rec[:], pv[qh][64:65, :])
                    rb = cp.tile([64, 512], F32, tag="rsbc", name="rb")
                    nc.gpsimd.partition_broadcast(rb[:], rec[:])
                    nc.vector.tensor_tensor(
                        attn_T[base:base + 64, dt_, qh * 512:(qh + 1) * 512],
                        pv[qh][0:64, :], rb[:], OP.mult)

        def residual_out(wname, src_T, bias_tile, res_T, out_T, stats_sb):
            """out_T = (W^T @ src_T) + bias + res_T ; accumulate sum/sumsq."""
            n_ki = src_T.shape[1]
            for dd in range(DT):
                blk = load_wblk(wname, dd)
                for th in range(TH):
                    pt = psB()
                    for ki in range(n_ki):
                        nc.tensor.matmul(pt[:], blk[:, ki],
                                         src_T[:, ki, th * 512:(th + 1) * 512],
                                         start=(ki == 0), stop=(ki == n_ki - 1))
                    dst = out_T[:, dd, th * 512:(th + 1) * 512]
                    c = dd * TH + th
                    nc.vector.scalar_tensor_tensor(
                        dst, pt[:], bias_tile[:, dd:dd + 1],
                        res_T[:, dd, th * 512:(th + 1) * 512],
                        OP.add, OP.add, accum_out=stats_sb[:, c:c + 1])
                    scr = wstage()
                    nc.vector.scalar_tensor_tensor(
                        scr[:], dst, 0.0, dst, OP.add, OP.mult,
                        accum_out=stats_sb[:, 8 + c:8 + c + 1])

        def stats_allreduce(stats_sb, name):
            pt = psA()
            nc.tensor.matmul(pt[:1, :16], onesf[:], stats_sb[:],
                             start=True, stop=True)
            red = sb.tile([1, 8], F32, tag=f"red_{name}")
            nc.vector.reduce_sum(red[:, 0:1], pt[0:1, 0:8],
                                 axis=mybir.AxisListType.X)
            nc.vector.reduce_sum(red[:, 1:2], pt[0:1, 8:16],
                                 axis=mybir.AxisListType.X)
            nc.vector.memset(red[:, 2:8], 0.0)
            ar_in = dram.tile([1, 8], F32, tag=f"ar_in_{name}")
            ar_out = dram.tile([1, 8], F32, tag=f"ar_out_{name}")
            nc.gpsimd.dma_start(ar_in[:], red[:])
            nc.gpsimd.collective_compute(
                "AllReduce", OP.add, replica_groups=[list(range(N_CORES))],
                ins=[ar_in.opt()], outs=[ar_out.opt()])
            g = sb.tile([1, 8], F32, tag=f"g_{name}")
            nc.sync.dma_start(g[:], ar_out[:])
            mu = sb.tile([1, 1], F32, tag=f"mu_{name}")
            nc.vector.tensor_scalar_mul(mu[:], g[:, 0:1], 1.0 / NTOT)
            ex2 = sb.tile([1, 1], F32, tag=f"ex2_{name}")
            nc.vector.tensor_scalar_mul(ex2[:], g[:, 1:2], 1.0 / NTOT)
            mu2 = sb.tile([1, 1], F32, tag=f"mu2_{name}")
            nc.vector.tensor_tensor(mu2[:], mu[:], mu[:], OP.mult)
            var = sb.tile([1, 1], F32, tag=f"var_{name}")
            nc.vector.tensor_tensor(var[:], ex2[:], mu2[:], OP.subtract)
            epst = sb.tile([1, 1], F32, tag=f"eps_{name}")
            nc.vector.memset(epst[:], EPS)
            std = sb.tile([1, 1], F32, tag=f"std_{name}")
            nc.scalar.activation(std[:], var[:], AF.Sqrt, bias=epst[:])
            rstd = sb.tile([1, 1], F32, tag=f"rstd_{name}")
            nc.vector.reciprocal(rstd[:], std[:])
            nmr = sb.tile([1, 1], F32, tag=f"nmr_{name}")
            nc.vector.tensor_tensor(nmr[:], mu[:], rstd[:], OP.mult)
            nc.vector.tensor_scalar_mul(nmr[:], nmr[:], -1.0)
            rstd_bc = sb.tile([128, 1], F32, tag=f"rstd_bc_{name}")
            nc.gpsimd.partition_broadcast(rstd_bc[:], rstd[:])
            nmr_bc = sb.tile([128, 1], F32, tag=f"nmr_bc_{name}")
            nc.gpsimd.partition_broadcast(nmr_bc[:], nmr[:])
            return rstd_bc, nmr_bc

        def materialize_norm(src_T, dst_T, rstd_bc, nmr_bc):
            for dd in range(DT):
                nc.vector.scalar_tensor_tensor(
                    dst_T[:, dd], src_T[:, dd], rstd_bc[:],
                    nmr_bc[:, :].to_broadcast([128, S]), OP.mult, OP.add)

        # ================= Phase 1: self attention =================
        q_T = sb.tile([128, DT, S], F32R, tag="g_q")
        k_T = sb.tile([128, DT, S], F32R, tag="g_k")
        v_tok = sb.tile([128, TT, H * 65], F32R, tag="g_v")
        attn_T = sb.tile([128, DT, S], F32R, tag="g_attn")

        project_fm("wq_m", x_T, q_T, bias_tile=bias["bq_m"])
        project_fm("wk_m", x_T, k_T, bias_tile=bias["bk_m"])
        project_v("wv_m", "bv_m", x_T, v_tok)
        attention(q_T, k_T, v_tok, attn_T, causal=True)

        r1_T = sb.tile([128, DT, S], F32R, tag="g_r1")
        stats1 = sb.tile([128, 16], F32, tag="stats1")
        residual_out("wo_m", attn_T, bias["bo_m"], x_T, r1_T, stats1)
        rstd1, nmr1 = stats_allreduce(stats1, "n1")

        # ================= Phase 2: cross attention =================
        qfix = sb.tile([128, DT], F32, tag="qfix")
        for dd in range(DT):
            nc.vector.scalar_tensor_tensor(
                qfix[:, dd:dd + 1], wsum_qc[:, dd:dd + 1], nmr1[:],
                bias["bq_c"][:, dd:dd + 1], OP.mult, OP.add)
        # q_raw matmul on r1 overlaps AllReduce #1; only the ACT fix waits
        project_fm("wq_c", r1_T, q_T, fix_tile=qfix, scale_ap=rstd1)
        project_fm("wk_c", enc_T, k_T, bias_tile=bias["bk_c"])
        project_v("wv_c", "bv_c", enc_T, v_tok)
        attention(q_T, k_T, v_tok, attn_T, causal=False)

        nmm_T = sb.tile([128, DT, S], F32, tag="g_x")      # reuses x_T space
        materialize_norm(r1_T, nmm_T, rstd1, nmr1)
        r2_T = sb.tile([128, DT, S], F32R, tag="g_enc")    # reuses enc_T space
        stats2 = sb.tile([128, 16], F32, tag="stats2")
        residual_out("wo_c", attn_T, bias["bo_c"], nmm_T, r2_T, stats2)
        rstd2, nmr2 = stats_allreduce(stats2, "n2")

        # ================= Phase 3: FFN =================
        ffix = sb.tile([128, FT], F32, tag="ffix")
        for ft in range(FT):
            nc.vector.scalar_tensor_tensor(
                ffix[:, ft:ft + 1], wsum_f1[:, ft:ft + 1], nmr2[:],
                bf1_sb[:, ft:ft + 1], OP.mult, OP.add)
        nmh_T = sb.tile([128, DT, S], F32, tag="g_r1")     # reuses r1_T space
        materialize_norm(r2_T, nmh_T, rstd2, nmr2)

        r3_T = sb.tile([128, DT, S], F32, tag="g_attn")    # reuses attn space
        stats3 = sb.tile([128, 16], F32, tag="stats3")
        h_T = sb.tile([128, FT, 512], F32R, tag="h_T")
        for th in range(TH):
            for ft in range(FT):
                blk = load_wblk("wf1", ft)
                pt = psB()
                for ki in range(DT):
                    nc.tensor.matmul(pt[:], blk[:, ki],
                                     r2_T[:, ki, th * 512:(th + 1) * 512],
                                     start=(ki == 0), stop=(ki == DT - 1))
                nc.scalar.activation(h_T[:, ft], pt[:], AF.Relu,
                                     bias=ffix[:, ft:ft + 1], scale=rstd2[:])
            for dd in range(DT):
                blk2 = load_wblk_ff2(dd)
                pt = psB()
                for ki in range(FT):
                    nc.tensor.matmul(pt[:], blk2[:, ki], h_T[:, ki],
                                     start=(ki == 0), stop=(ki == FT - 1))
                dst = r3_T[:, dd, th * 512:(th + 1) * 512]
                c = dd * TH + th
                nc.vector.scalar_tensor_tensor(
                    dst, pt[:], bias["bf2"][:, dd:dd + 1],
                    nmh_T[:, dd, th * 512:(th + 1) * 512], OP.add, OP.add,
                    accum_out=stats3[:, c:c + 1])
                scr = wstage()
                nc.vector.scalar_tensor_tensor(
                    scr[:], dst, 0.0, dst, OP.add, OP.mult,
                    accum_out=stats3[:, 8 + c:8 + c + 1])

        # transpose r3 to token-major (overlaps AllReduce #3)
        r3_tok = sb.tile([128, TT, D], F32, tag="g_q")     # reuses q space
        for tt in range(TT):
            for dd in range(DT):
                transpose_block(r3_tok[:, tt, dd * 128:(dd + 1) * 128],
                                r3_T[:, dd, tt * 128:(tt + 1) * 128])

        rstd3, nmr3 = stats_allreduce(stats3, "n3")
        for tt in range(TT):
            nc.vector.scalar_tensor_tensor(
                r3_tok[:, tt], r3_tok[:, tt], rstd3[:],
                nmr3[:, :].to_broadcast([128, D]), OP.mult, OP.add)
            nc.sync.dma_start(
                out_d.rearrange("(tt p) d -> p tt d", p=128)[:, tt],
                r3_tok[:, tt])


_NC_CACHE = {}


def kernel(**inputs):
    if "nc" not in _NC_CACHE:
        _NC_CACHE["nc"] = build_nc()
    nc = _NC_CACHE["nc"]
    in_maps = []
    for b in range(N_CORES):
        m = {"data_dec": np.ascontiguousarray(
                 np.asarray(inputs["data_dec"], dtype=np.float32)[b]),
             "encoder_out": np.ascontiguousarray(
                 np.asarray(inputs["encoder_out"], dtype=np.float32)[b])}
        for k, v in inputs.items():
            if k not in ("data_dec", "encoder_out"):
                m[k] = np.ascontiguousarray(np.asarray(v, dtype=np.float32))
        in_maps.append(m)
    res = bass_utils.run_bass_kernel_spmd(nc, in_maps,
                                          core_ids=list(range(N_CORES)))
    return np.stack([res.results[b]["out"] for b in range(N_CORES)], axis=0)
